# revision 21
# baseline (speedup 1.0000x reference)
"""BandSplit (gather -> per-band MLP -> scatter-add OLA -> /ola) on 8 TRN2 cores.

Strategy
--------
The whole reference computation is linear in x (the per-band pre/post weights,
melbank weights, mask, scatter-add and the final /ola are all linear maps, and
the biases contribute an x-independent constant).  On the host we fold all of
it into a single matrix A of shape (C*F, C*F) mapping the (c, f) spectrum of
one (b, t) token to the (c, f) output spectrum:

    out[b, :, t, :] = A^T @ vec(x[b, :, t, :]) + const

Because every mel band covers a *contiguous* frequency range of width <= Wmax,
A is block-banded: A[(ci, fi), (co, fo)] == 0 unless |fi - fo| < Wmax.  The
device kernel is therefore a banded matmul, data-parallel over the 4096
(b, t) tokens across the 8 NeuronCores (512 tokens/core) with zero
cross-core communication.  The bias constant is folded into a spare padded
row of A (row F, with x padded so column F == 1.0).

Per core: load packed A band tiles (bf16) + its x slice (f32), cast x to bf16
on the vector engine, PE-transpose x tiles to put f on partitions, run banded
bf16 matmuls (fp32 PSUM accumulate), drain to SBUF on the vector engine, DMA
out.  bf16 operands take one PE pass per matmul (fp32 takes two) and enable
the fast weight-load path.
"""

import numpy as np

_P = 128


def _fold_matrix(pre_w, pre_b, post_w, post_b, idx, melw, mask, ola_window):
    """Fold the full reference computation into (A, const).

    A: (C, F, C, F) with out[co, fo] = sum_{ci, fi} x[ci, fi] * A[ci, fi, co, fo]
    const: (C, F) additive constant from the biases.
    """
    K, W = idx.shape
    C = 2
    F = ola_window.shape[0]

    pre_w = np.asarray(pre_w, np.float64)
    post_w = np.asarray(post_w, np.float64)
    pre_b = np.asarray(pre_b, np.float64)
    post_b = np.asarray(post_b, np.float64)
    wts = (np.asarray(melw, np.float64) * np.asarray(mask, np.float64))
    msk = np.asarray(mask, np.float64)
    idx = np.asarray(idx)

    # Per-band folded linear map: M[k, i=(w,cin), j=(w',cout)]
    M = np.einsum('kio,koj->kij', pre_w, post_w).reshape(K, W, C, W, C)
    vals = M * wts[:, :, None, None, None] * msk[:, None, None, :, None]

    fin = idx[:, :, None, None, None].astype(np.int64)
    fout = idx[:, None, None, :, None].astype(np.int64)
    cin = np.arange(C)[None, None, :, None, None]
    cout = np.arange(C)[None, None, None, None, :]
    flat = ((cin * F + fin) * C + cout) * F + fout
    A = np.bincount(
        np.broadcast_to(flat, vals.shape).ravel(), weights=vals.ravel(),
        minlength=C * F * C * F,
    ).reshape(C, F, C, F)
    A /= ola_window[None, None, None, :]

    # Bias constant: (sum_o pre_b[k,o] * post_w[k,o,(w',co)] + post_b) * mask, /ola
    bv = (np.einsum('ko,koj->kj', pre_b, post_w) + post_b).reshape(K, W, C)
    bv = bv * msk[:, :, None]
    cflat = (np.arange(C)[None, None, :] * F + idx[:, :, None]).astype(np.int64)
    const = np.bincount(
        np.broadcast_to(cflat, bv.shape).ravel(), weights=bv.ravel(),
        minlength=C * F,
    ).reshape(C, F)
    const /= ola_window[None, :]
    return A, const


_PROGRAM_CACHE = {}


def _build_program(C, F_PAD, KI, T_CORE, offs, TW, wins, n_cores):
    """Build the Bass/Tile program. Returns the compiled Bass object."""
    import concourse.bass as bass
    import concourse.tile as tile
    import concourse.mybir as mybir
    from concourse import bacc
    from concourse.masks import make_identity

    f32 = mybir.dt.float32
    bf16 = mybir.dt.bfloat16
    P = _P
    TCH = T_CORE // P

    nc = bacc.Bacc("TRN2", target_bir_lowering=False, debug=False,
                   num_devices=n_cores)
    # xs is [TCH, C, P, F_PAD] so each token chunk is one contiguous DMA
    xs = nc.dram_tensor("xs", [TCH, C, P, F_PAD], f32, kind="ExternalInput")
    # ab is packed band windows, laid out [P, TW] (ki, ci, co at offsets offs)
    ab = nc.dram_tensor("ab", [P, TW], bf16, kind="ExternalInput")
    y = nc.dram_tensor("y", [C, T_CORE, 1025], f32, kind="ExternalOutput")
    F_OUT = 1025

    # Per (co, bank) ordered list of matmul segments for PSUM accumulate flags.
    # Emission order: ki, ci, co, seg.
    BANKS = [(b * 512, min(F_PAD, (b + 1) * 512)) for b in range((F_PAD + 511) // 512)]

    def segments(ki):
        lo, hi = wins[ki]
        segs = []
        for b, (bs, be) in enumerate(BANKS):
            s, e = max(lo, bs), min(hi, be)
            if s < e:
                segs.append((b, s, e))
        return segs

    touches = {}
    for ki in range(KI):
        for ci in range(C):
            for co in range(C):
                for (b, s, e) in segments(ki):
                    touches.setdefault((co, b), []).append((ki, ci, s, e))

    GRP = 3                      # ki per transpose group (one ACT drain each)
    NG = (KI + GRP - 1) // GRP   # groups per channel

    with tile.TileContext(nc) as tc:
        with (
            tc.tile_pool(name="apool", bufs=1) as apool,
            tc.tile_pool(name="xbpool", bufs=4) as xbpool,
            tc.tile_pool(name="xtpool", bufs=2) as xtpool,
            tc.tile_pool(name="opool", bufs=2) as opool,
            tc.tile_pool(name="idpool", bufs=1) as idpool,
            tc.tile_pool(name="pspool", bufs=1, space="PSUM") as pspool,
            tc.tile_pool(name="tpspool", bufs=2, space="PSUM") as tpspool,
        ):
            xbf = {}

            def load_x(tch, split=1):
                # SWDGE cast-during-DMA: f32 DRAM -> bf16 SBUF, one per channel
                for ci in range(C):
                    t = xbpool.tile([P, F_PAD], bf16, tag=f"xbf_{ci}",
                                    name=f"xbf_{tch}_{ci}")
                    step = F_PAD // split
                    for s in range(split):
                        nc.gpsimd.dma_start(
                            t[:, s * step:(s + 1) * step],
                            xs[tch, ci, :, s * step:(s + 1) * step])
                    xbf[(tch, ci)] = t

            ident = idpool.tile([P, P], bf16, name="ident")
            make_identity(nc, ident[:])

            # kick off all x loads up front (xbpool holds every chunk)
            load_x(0, split=2)
            for t_ in range(1, TCH):
                load_x(t_)

            # warm up the PE clock gate (HAM) with throwaway matmuls while the
            # DMAs fill SBUF; output goes to the pt_0 slot which the first real
            # accumulation group overwrites (start=True clears the bank)
            warm = pspool.tile([P, F_PAD], f32, tag="pt_0", name="warm")
            for _ in range(20):
                nc.tensor.matmul(warm[:, :P], ident[:], ident[:],
                                 start=True, stop=True)

            # A band tiles: one resident SBUF slab, 3 contiguous DMAs
            abig = apool.tile([P, TW], bf16, name="abig")
            bounds = [offs[(k0, 0, 0)] for k0 in range(0, KI, 3)] + [TW]
            for i in range(len(bounds) - 1):
                nc.sync.dma_start(abig[:, bounds[i]:bounds[i + 1]],
                                  ab[:, bounds[i]:bounds[i + 1]])

            def a_tile(ci, co, ki):
                o = offs[(ki, ci, co)]
                return abig[:, o:o + wins[ki][1] - wins[ki][0]]

            xt = {}

            def _drain(eng, dst, src):
                if eng == "scalar":
                    nc.scalar.copy(dst, src)
                else:
                    nc.vector.tensor_copy(dst, src)

            def transpose_ops(tch, fine=False, engines=("scalar",)):
                """Thunks: per (ci, ki-group) — or per ki when fine — transpose
                + PSUM->SBUF drain for chunk tch."""
                for ci in range(C):
                    for g in range(NG):
                        kis = list(range(g * GRP, min(KI, (g + 1) * GRP)))
                        n = len(kis)
                        state = {}

                        def begin(tch, ci, g, state):
                            if 'tps' not in state:
                                state['tps'] = tpspool.tile(
                                    [P, GRP * P], bf16, tag="tps",
                                    name=f"tps_{tch}_{ci}_{g}")
                                state['xtt'] = xtpool.tile(
                                    [P, GRP, P], bf16, tag=f"xt_{ci}_{g}",
                                    name=f"xt_{tch}_{ci}_{g}")
                                xt[(tch, ci, g)] = state['xtt']

                        if fine:
                            for j, ki in enumerate(kis):
                                def op(tch=tch, ci=ci, g=g, j=j, ki=ki, state=state,
                                       eng=engines[(ci * KI + g * GRP + j) % len(engines)]):
                                    begin(tch, ci, g, state)
                                    nc.tensor.transpose(
                                        state['tps'][:, j * P:(j + 1) * P],
                                        xbf[(tch, ci)][:, ki * P:(ki + 1) * P], ident[:])
                                    _drain(eng, state['xtt'][:, j],
                                           state['tps'][:, j * P:(j + 1) * P])
                                yield op
                        else:
                            def op(tch=tch, ci=ci, g=g, kis=kis, n=n, state=state,
                                   eng=engines[(ci * NG + g) % len(engines)]):
                                begin(tch, ci, g, state)
                                for j, ki in enumerate(kis):
                                    nc.tensor.transpose(
                                        state['tps'][:, j * P:(j + 1) * P],
                                        xbf[(tch, ci)][:, ki * P:(ki + 1) * P], ident[:])
                                _drain(eng, state['xtt'][:, :n], state['tps'][:, :n * P])
                            yield op

            # chunk 0 transposes: fine-grained, alternating drain engines, so
            # the first matmuls unblock as early as possible
            t0ops = list(transpose_ops(0, fine=True, engines=("vector", "scalar")))
            for i, op in enumerate(t0ops):
                op()
                if i == 5:
                    # keep the PE clock warm while chunk-0 data finishes landing
                    for _ in range(16):
                        nc.tensor.matmul(warm[:, :P], ident[:], ident[:],
                                         start=True, stop=True)

            for tch in range(TCH):
                nxt = list(transpose_ops(tch + 1)) if tch + 1 < TCH else []

                pt = {}
                for co in range(C):
                    pt[co] = pspool.tile([P, F_PAD], f32, tag=f"pt_{co}",
                                         name=f"pt_{tch}_{co}")
                # interleave next-chunk transposes into the matmul stream so the
                # PE never idles long enough for HAM to re-throttle
                for ki in range(KI):
                    if ki % 2 == 1 and nxt:
                        nxt.pop(0)()
                    lo = wins[ki][0]
                    for ci in range(C):
                        lhsT = xt[(tch, ci, ki // GRP)][:, ki % GRP, :]
                        for co in range(C):
                            for (b, s, e) in segments(ki):
                                order = touches[(co, b)]
                                first = order[0] == (ki, ci, s, e)
                                last = order[-1] == (ki, ci, s, e)
                                nc.tensor.matmul(
                                    pt[co][:, s:e],
                                    lhsT,
                                    a_tile(ci, co, ki)[:, s - lo:e - lo],
                                    start=first, stop=last,
                                )
                for op in nxt:
                    op()

                # drain PSUM -> SBUF -> HBM (split copies for finer overlap)
                ot = opool.tile([P, C, F_OUT], f32, tag="out", name=f"out_{tch}")
                for co in range(C):
                    nc.vector.tensor_copy(ot[:, co, :512], pt[co][:, :512])
                    nc.vector.tensor_copy(ot[:, co, 512:F_OUT], pt[co][:, 512:F_OUT])
                    nc.sync.dma_start(y[co, tch * P:(tch + 1) * P, :], ot[:, co])

    nc.compile()
    return nc


def kernel(**inputs):
    import ml_dtypes

    x = np.ascontiguousarray(np.asarray(inputs["x"], np.float32))
    B, C, T, F = x.shape
    assert (B, C, F) == (4, 2, 1025), (B, C, F)
    N_CORES = 8
    T_CORE_TOK = B * (T // N_CORES)          # tokens per core

    A, const = _fold_matrix(
        inputs["pre_w"], inputs["pre_b"], inputs["post_w"], inputs["post_b"],
        inputs["idx"], inputs["melw"], inputs["mask"], inputs["ola_window"],
    )

    KI = (F + _P - 1) // _P                   # 9 f-chunks of 128
    F_PAD = KI * _P                           # 1152

    # padded A, with the bias constant folded into spare row F (ci = 0)
    Apad = np.zeros((C, F_PAD, C, F_PAD), np.float32)
    Apad[:, :F, :, :F] = A.astype(np.float32)
    Apad[0, F, :, :F] = const.astype(np.float32)

    # exact nonzero column window per 128-row chunk (same for all channel blocks)
    nz = (Apad != 0).any(axis=(0, 2))          # (F_PAD rows, F_PAD cols)
    wins = []
    for ki in range(KI):
        cols = nz[ki * _P:(ki + 1) * _P].any(axis=0)
        nzc = np.nonzero(cols)[0]
        if len(nzc) == 0:
            lo, hi = ki * _P, ki * _P + 1
        else:
            lo, hi = int(nzc[0]), int(nzc[-1]) + 1
        wins.append((lo, hi))
    # coverage: every output column [0, F) must be written by >= 1 matmul
    covered = np.zeros(F_PAD, bool)
    for lo, hi in wins:
        covered[lo:hi] = True
    assert covered[:F].all(), "window coverage hole"

    # packed band layout: offsets per (ki, ci, co), width = window width
    offs = {}
    tw = 0
    for ki in range(KI):
        w = (wins[ki][1] - wins[ki][0] + 15) // 16 * 16
        for ci in range(C):
            for co in range(C):
                offs[(ki, ci, co)] = tw
                tw += w
    TW = tw

    ab = np.zeros((_P, TW), ml_dtypes.bfloat16)
    for ki in range(KI):
        lo, hi = wins[ki]
        for ci in range(C):
            for co in range(C):
                o = offs[(ki, ci, co)]
                ab[:, o:o + hi - lo] = Apad[ci, ki * _P:(ki + 1) * _P, co, lo:hi]

    key = (C, F_PAD, KI, T_CORE_TOK, TW, tuple(wins), N_CORES)
    if key not in _PROGRAM_CACHE:
        _PROGRAM_CACHE[key] = _build_program(C, F_PAD, KI, T_CORE_TOK, offs, TW, wins, N_CORES)
    nc = _PROGRAM_CACHE[key]

    # shard: core m gets t in [m*T/8, (m+1)*T/8), tokens ordered (b, t_local)
    TS = T // N_CORES
    TCH = T_CORE_TOK // _P
    in_maps = []
    for m in range(N_CORES):
        xs_m = np.zeros((TCH, C, _P, F_PAD), np.float32)
        sl = x[:, :, m * TS:(m + 1) * TS, :]             # (B, C, TS, F)
        tok = sl.transpose(1, 0, 2, 3).reshape(C, T_CORE_TOK, F)
        xs_m[:, :, :, :F] = tok.reshape(C, TCH, _P, F).transpose(1, 0, 2, 3)
        xs_m[:, :, :, F] = 1.0                            # bias row
        in_maps.append({"xs": xs_m, "ab": ab})

    from concourse.bass_utils import run_bass_kernel_spmd
    res = run_bass_kernel_spmd(nc, in_maps, core_ids=list(range(N_CORES)))
    globals()["_LAST_RESULT"] = res

    out = np.empty((B, C, T, F), np.float32)
    for m in range(N_CORES):
        ym = res.results[m]["y"].reshape(C, B, TS, F)
        out[:, :, m * TS:(m + 1) * TS, :] = ym.transpose(1, 0, 2, 3)
    return out
